# revision 21
# baseline (speedup 1.0000x reference)
"""Trainium2 Bass kernel for flax MultiHeadDotProductAttention.

Shapes (hardcoded): B=4, Q=K=1500, D=1024, H=16, HD=64.
Sharding: 8 cores = 4 batches x 2 head-groups (8 heads each).
Each core computes its batch's attention output for its 8 heads plus the
output projection restricted to those heads; the host sums the two
head-group partials per batch and adds bo.

Dataflow per core (all layouts chosen so no on-device transposes are
needed; host passes x pre-transposed):
  qT/kT [hhd, seq] and v [seq, hhd] via projection matmuls;
  S^T[k,q] = kT.T-slices @ qT (K=64; the two head matmuls of a pair run
  CONCURRENTLY on the two row-halves of the PE array via auto-derived
  tile_position);
  P^T = exp(S^T/8) on ScalarE (psum->sbuf, bf16);
  attn_outT += v_tile.T @ P^T (bf16) with a ones-column in V so row 64
  accumulates the softmax denominator; normalization via a selector
  matmul broadcast + approximate reciprocal; out-projection consumes
  the normalized [hhd, q] tiles as stationary operands -> natural
  [q, d] output tiles DMA'd straight to HBM.

SCHEDULE (v3): two serial resources dominate: ScalarE's exp (~155us:
144 steps x ~1.07us) and the DMA fabric's ~300ns/descriptor issue rate.
  - All inputs are HOST-PACKED so each tensor loads as 128 fat
    descriptors (one per partition): x as [128, DCH*SEQ] (24KB/desc),
    wq|wk|wv fused as [128, DCH*3*512], wo as [128, 4*1024]. Inputs
    land in ~15us instead of ~45us. Output rows are written whole
    (4KB descriptors). Constants are memset, not DMA'd.
  - Minimal prologue (kT block 0 + qT block 0 chunk 0) so the first
    exp issues ~15us in; all other projection groups and the previous
    chunk's out-projection are metered into the attention steps by a
    deadline scheduler, riding ScalarE's slack.
  - Each pair's normalize is deferred into the next pair's first steps
    so the in-order PE queue never makes ScalarE wait at a boundary.
  - Filler groups are group-atomic over a 2-bank aux psum ring and
    drained before each normalize, so the PE queue cannot head-block.
"""

import os
import sys

sys.path.insert(0, "/opt/trn_rl_repo")

import numpy as np  # noqa: E402
import ml_dtypes  # noqa: E402
import concourse.bacc as bacc  # noqa: E402
import concourse.mybir as mybir  # noqa: E402
import concourse.tile as tile  # noqa: E402
from concourse.bass_utils import run_bass_kernel_spmd  # noqa: E402

F32 = mybir.dt.float32
F32R = mybir.dt.float32r
BF16 = mybir.dt.bfloat16
AF = mybir.ActivationFunctionType

B, SEQ, D, H, HD = 4, 1500, 1024, 16, 64
HG = 8                      # heads per group
HHD = HG * HD               # 512
DCH = D // 128              # 8 d-chunks
HB = HHD // 128             # 4 hhd blocks (2 heads each)
NPAIR = HB                  # 4 head pairs per group
QC = [(0, 512), (512, 512), (1024, 476)]          # q chunks
KT = [(i * 128, min(128, SEQ - i * 128)) for i in range((SEQ + 127) // 128)]
NKT = len(KT)               # 12 (last tile 92 rows)
LAG = 4                             # attn@V trails exp by LAG steps
GTOT = len(QC) * NPAIR * NKT + LAG  # flat pipeline steps

MODE = os.environ.get("BASS_MM_DTYPE", "bf16")


class _Sched:
    """Deadline-driven, group-atomic filler scheduler.

    Groups are lists of parcels (closures, each ~2 matmuls). At most one
    group is mid-flight at any time, so the shared 2-bank psum ring the
    parcels allocate from can never deadlock the in-order PE queue.
    """

    def __init__(self):
        self.pending = []          # [ (dl, seq, earliest, parcels) ]
        self.cur = None            # [dl, parcels_remaining]
        self.seq = 0

    def add(self, parcels, earliest, dl):
        self.pending.append((dl, self.seq, earliest, list(parcels)))
        self.seq += 1
        self.pending.sort()

    def _take_next(self, g, force):
        for i, (dl, _, earliest, parcels) in enumerate(self.pending):
            must = dl - len(parcels) < g
            if earliest <= g or must or force:
                self.pending.pop(i)
                self.cur = [dl, parcels]
                return True
        return False

    def on_step(self, g, budget=1):
        emitted = 0
        while True:
            if self.cur is None and not self._take_next(g, force=False):
                return
            dl, parcels = self.cur
            while parcels:
                latest = dl - (len(parcels) - 1)
                if latest <= g or emitted < budget:
                    parcels.pop(0)()
                    emitted += 1
                else:
                    return
            self.cur = None

    def flush_cur(self):
        if self.cur is not None:
            for p in self.cur[1]:
                p()
            self.cur = None

    def drain(self, g):
        self.flush_cur()
        while self.pending or self.cur:
            if self.cur is None:
                self._take_next(g, force=True)
            self.flush_cur()


def _build(mode, with_bias):
    mt = BF16 if mode == "bf16" else F32R          # big-matmul operand dtype
    MTD = BF16 if mode == "bf16" else F32          # dram dtype for x/w/b

    nc = bacc.Bacc("TRN2", target_bir_lowering=False, debug=False, num_devices=8)

    # host-packed layouts: one fat descriptor per partition; x is packed
    # per q-chunk block so the first-needed columns land first
    xq_ds, xkv_ds = [], []
    for qci, (qo, cwq) in enumerate(QC):
        xq_ds.append(nc.declare_dram_parameter(
            f"xqp{qci}", [128, DCH * cwq], MTD, isOutput=False))
        xkv_ds.append(nc.declare_dram_parameter(
            f"xkvp{qci}", [128, DCH * cwq], MTD, isOutput=False))
    wqkv_d = nc.declare_dram_parameter("wqkv", [128, DCH * 3 * HHD], MTD,
                                       isOutput=False)
    wo_d = nc.declare_dram_parameter("wop", [128, HB * D], MTD, isOutput=False)
    bq_d = nc.declare_dram_parameter("bq", [1, HHD], MTD, isOutput=False)
    bk_d = nc.declare_dram_parameter("bk", [1, HHD], MTD, isOutput=False)
    bv_d = nc.declare_dram_parameter("bv", [1, HHD], MTD, isOutput=False)
    out_d = nc.declare_dram_parameter("out", [SEQ, D], F32, isOutput=True)

    def mcast(ap):
        # view a dram param as the matmul dtype
        return ap if mode == "bf16" else ap.bitcast(F32R)

    with tile.TileContext(nc) as tc:
        from contextlib import ExitStack

        with ExitStack() as ctx:
            ctx.enter_context(nc.allow_low_precision(
                reason="bf16/f32r matmul operands; psum accumulation is fp32"
            ))
            const = ctx.enter_context(tc.tile_pool(name="const", bufs=1))
            ones_r = const.tile([1, 512], mt, tag="ones")
            nc.vector.memset(ones_r[:], 1.0)
            # selector: den staging row 64 -> all 64 output partitions.
            # F32R tiles can't be memset directly; build in F32 and copy
            # (the copy rounds to F32r, which the PE requires of producers)
            zstage = const.tile([128, 512], F32, tag="zstage")
            nc.vector.memset(zstage[:], 0.0)
            sel_sb = const.tile([128, 64], F32R, tag="sel")
            ds_e = const.tile([128, 512], F32R, tag="dse")
            ds_o = const.tile([128, 512], F32R, tag="dso")
            nc.vector.tensor_copy(sel_sb[:], zstage[:, 0:64])
            nc.vector.tensor_copy(ds_e[:], zstage[:])
            nc.vector.tensor_copy(ds_o[:], zstage[:])
            nc.vector.memset(zstage[64:65, 0:64], 1.0)
            nc.vector.tensor_copy(sel_sb[64:65, :], zstage[64:65, 0:64])
            bq_sb = const.tile([1, HHD], mt, tag="bq")
            bk_sb = const.tile([1, HHD], mt, tag="bk")
            bv_sb = const.tile([1, HHD], mt, tag="bv")
            if with_bias:
                nc.sync.dma_start(bq_sb[:], mcast(bq_d[:]))
                nc.sync.dma_start(bk_sb[:], mcast(bk_d[:]))
                nc.sync.dma_start(bv_sb[:], mcast(bv_d[:]))

            # persistent activations for the attention phase
            qT_b = []
            for i in range(HB):
                qT_b.append(const.tile([128, SEQ], mt, tag=f"qT{i}",
                                       name=f"qT{i}"))  # [hhd%128, q] per block
            kT = const.tile([128, HB, SEQ], mt, tag="kT")
            # v: one tile per k-tile ([k%128, head, hd|1]); the 65th column is
            # ones so attn@V also accumulates the softmax denominator in row 64
            v_t = []
            for kt in range(NKT):
                vt = const.tile([128, HG, 65], BF16, tag=f"v{kt}", name=f"v{kt}")
                nc.vector.memset(vt[:, :, 64:65], 1.0)
                v_t.append(vt)

            # ---- packed input tiles; DMA order = arrival order:
            # wqkv half A, xkv qc0 (first kT group + first v groups),
            # wqkv half B, xq qc0 (qT block 0 -> first exp), then the rest
            xkv_ts, xq_ts = [], []
            for qci, (qo, cwq) in enumerate(QC):
                xkv_ts.append(const.tile([128, DCH, cwq], mt, tag=f"xkv{qci}",
                                         name=f"xkv{qci}"))
                xq_ts.append(const.tile([128, DCH, cwq], mt, tag=f"xq{qci}",
                                        name=f"xq{qci}"))
            wqkv_t = const.tile([128, DCH, 3, HHD], mt, tag="wqkv")
            wo_sb = const.tile([128, HB, D], mt, tag="wo")
            wqkv_d4 = wqkv_d.rearrange("p (c t n) -> p c t n", c=DCH, t=3)

            def dma_x(ts, ds, qci):
                nc.sync.dma_start(
                    ts[qci][:],
                    mcast(ds[qci].rearrange("p (c s) -> p c s", c=DCH)))

            nc.sync.dma_start(wqkv_t[:, 0:4], mcast(wqkv_d4[:, 0:4]))
            dma_x(xkv_ts, xkv_ds, 0)
            nc.sync.dma_start(wqkv_t[:, 4:8], mcast(wqkv_d4[:, 4:8]))
            dma_x(xq_ts, xq_ds, 0)
            dma_x(xkv_ts, xkv_ds, 1)
            dma_x(xkv_ts, xkv_ds, 2)
            dma_x(xq_ts, xq_ds, 1)
            dma_x(xq_ts, xq_ds, 2)
            nc.sync.dma_start(wo_sb[:],
                              mcast(wo_d.rearrange("p (c n) -> p c n", c=HB)))

            def x_slice(ts, c, qo2, cw2):
                # [c][qc-block] view of a column range (must lie in one block)
                for qci, (qo, cwq) in enumerate(QC):
                    if qo <= qo2 and qo2 + cw2 <= qo + cwq:
                        return ts[qci][:, c, qo2 - qo:qo2 - qo + cw2]
                raise AssertionError((qo2, cw2))

            wq_sb = [wqkv_t[:, c, 0, :] for c in range(DCH)]
            wk_sb = [wqkv_t[:, c, 1, :] for c in range(DCH)]
            wv_sb = [wqkv_t[:, c, 2, :] for c in range(DCH)]

            # ---------------- psum pools (8 banks total) ----------------
            st_ps = ctx.enter_context(tc.tile_pool(name="stps", bufs=2,
                                                   space="PSUM"))    # 4 banks
            at_ps = ctx.enter_context(tc.tile_pool(name="atps", bufs=2,
                                                   space="PSUM"))    # 2 banks
            aux_ps = ctx.enter_context(tc.tile_pool(name="auxps", bufs=2,
                                                    space="PSUM"))   # 2 banks
            p_pool = ctx.enter_context(tc.tile_pool(name="p", bufs=20))
            opart = ctx.enter_context(tc.tile_pool(name="opart", bufs=8))
            an_pool = ctx.enter_context(tc.tile_pool(name="an", bufs=8))
            small = ctx.enter_context(tc.tile_pool(name="small", bufs=4))

            # ---------------- split-emission filler groups ----------------

            def projT_parcels(dst2d, w_sb, b_sb, xs, hb, qci, per=2):
                """qT/kT projection group -> [hhd%128, q-chunk], split.
                xs is the per-qc-block tile list (xq_ts or xkv_ts)."""
                qo2, cw2 = QC[qci]
                hold = {}
                parcels = []
                cs = list(range(0, DCH, per))
                for pi, c0 in enumerate(cs):
                    last = pi == len(cs) - 1

                    def emit(c0=c0, last=last):
                        if c0 == 0:
                            hold['ps'] = aux_ps.tile(
                                [128, 512], F32, tag="aux", bufs=2,
                                name=f"tps{hb}_{qci}")
                        ps = hold['ps']
                        for c in range(c0, min(c0 + per, DCH)):
                            nc.tensor.matmul(
                                ps[:, :cw2],
                                w_sb[c][:, hb * 128:(hb + 1) * 128],
                                x_slice(xs, c, qo2, cw2),
                                start=(c == 0),
                                stop=(not with_bias and c == DCH - 1),
                            )
                        if last:
                            if with_bias:
                                nc.tensor.matmul(
                                    ps[:, :cw2],
                                    b_sb[0:1, hb * 128:(hb + 1) * 128],
                                    ones_r[0:1, :cw2],
                                    start=False, stop=True,
                                )
                            nc.vector.tensor_copy(dst2d[:, qo2:qo2 + cw2],
                                                  ps[:, :cw2])
                    parcels.append(emit)
                return parcels

            def projV_parcels(kt, per=2):
                """v projection group for k-tile kt -> v_t[kt], split."""
                ko, kh = KT[kt]
                hold = {}
                parcels = []
                cs = list(range(0, DCH, per))
                for pi, c0 in enumerate(cs):
                    last = pi == len(cs) - 1

                    def emit(c0=c0, last=last):
                        if c0 == 0:
                            hold['ps'] = aux_ps.tile(
                                [128, 512], F32, tag="aux", bufs=2,
                                name=f"vps{kt}")
                        ps = hold['ps']
                        for c in range(c0, min(c0 + per, DCH)):
                            nc.tensor.matmul(
                                ps[:kh, :],
                                x_slice(xkv_ts, c, ko, kh),
                                wv_sb[c][:, :],
                                start=(c == 0),
                                stop=(not with_bias and c == DCH - 1),
                            )
                        if last:
                            if with_bias:
                                nc.tensor.matmul(
                                    ps[:kh, :],
                                    ones_r[0:1, :kh],
                                    bv_sb[0:1, :],
                                    start=False, stop=True,
                                )
                            nc.vector.tensor_copy(
                                v_t[kt][:kh, :, 0:64],
                                ps[:kh, :].rearrange("p (h c) -> p h c", c=64),
                            )
                    parcels.append(emit)
                return parcels

            def outproj_parcels(qci, anorms):
                """out-projection of chunk qci; two parcels per 128-row
                sub-block (one per d-half), one whole-row DMA at the end."""
                qo, cw = QC[qci]
                parcels = []
                for s in range((cw + 127) // 128):
                    sw = min(128, cw - s * 128)
                    hold = {}
                    for dc in range(2):
                        for jh in range(2):
                            def emit(s=s, sw=sw, dc=dc, jh=jh):
                                if dc == 0 and jh == 0:
                                    hold['osb'] = small.tile(
                                        [128, 1024], F32, tag="os", bufs=2,
                                        name=f"osb{qci}_{s}")
                                if jh == 0:
                                    hold['op'] = aux_ps.tile(
                                        [128, 512], F32, tag="aux", bufs=2,
                                        name=f"op{qci}_{s}_{dc}")
                                op = hold['op']
                                for j in (0, 1) if jh == 0 else (2, 3):
                                    nc.tensor.matmul(
                                        op[:sw, :],
                                        anorms[j][:, s * 128:s * 128 + sw],
                                        wo_sb[:, j, dc * 512:(dc + 1) * 512],
                                        start=(j == 0), stop=(j == NPAIR - 1),
                                    )
                                if jh == 1:
                                    nc.vector.tensor_copy(
                                        hold['osb'][:sw,
                                                    dc * 512:(dc + 1) * 512],
                                        op[:sw, :])
                                if dc == 1 and jh == 1:
                                    nc.sync.dma_start(
                                        out_d[qo + s * 128:
                                              qo + s * 128 + sw, :],
                                        hold['osb'][:sw, :],
                                    )
                            parcels.append(emit)
                return parcels

            def outpart_parcels(qci, anorms, parts):
                """last chunk: pre-accumulate out-proj over pairs 0-1 into
                SBUF so the post-exp tail only runs j=2,3 + add + DMA."""
                qo, cw = QC[qci]
                for si in range((cw + 127) // 128):
                    sw = min(128, cw - si * 128)
                    for dc in range(2):
                        def emit(si=si, sw=sw, dc=dc):
                            op = aux_ps.tile([128, 512], F32, tag="aux",
                                             bufs=2, name=f"pp{si}_{dc}")
                            for j in (0, 1):
                                nc.tensor.matmul(
                                    op[:sw, :],
                                    anorms[j][:, si * 128:si * 128 + sw],
                                    wo_sb[:, j, dc * 512:(dc + 1) * 512],
                                    start=(j == 0), stop=(j == 1),
                                )
                            pt = opart.tile([128, 512], F32, tag="op",
                                            bufs=8, name=f"pt{si}_{dc}")
                            nc.vector.tensor_copy(pt[:sw, :], op[:sw, :])
                            parts[(si, dc)] = pt
                        yield emit

            # ---------------- filler schedule ----------------
            # flat pipeline step index: u = pair_index*12 + ktile
            sched = _Sched()
            # v projections: v[kt] consumed by attn@V at step kt+LAG
            for kt in range(NKT):
                sched.add(projV_parcels(kt, per=2), max(0, kt - 2),
                          kt + LAG - 1)
            # kT blocks 1-3: block hb's qc slice first consumed by scores of
            # pair hb at step 12*hb + 4*qc
            for hb in range(1, HB):
                for qci in range(len(QC)):
                    sched.add(projT_parcels(kT[:, hb, :], wk_sb, bk_sb,
                                            xkv_ts, hb, qci, per=2),
                              0, 12 * hb + 4 * qci - 1)
            # qT blocks: block j chunk qci consumed from step 12*(4*qci+j)
            for j in range(1, HB):
                sched.add(projT_parcels(qT_b[j], wq_sb, bq_sb, xq_ts, j, 0,
                                        per=2),
                          0, 12 * j - 1)
            for qci in range(1, len(QC)):
                for j in range(HB):
                    sched.add(projT_parcels(qT_b[j], wq_sb, bq_sb, xq_ts, j,
                                            qci, per=2),
                              12 * (4 * qci + j) - 16,
                              12 * (4 * qci + j) - 1)

            # kT block 0 chunks 1-2 ride as early fillers (their x blocks
            # land just in time)
            for qci in (1, 2):
                sched.add(projT_parcels(kT[:, 0, :], wk_sb, bk_sb, xkv_ts,
                                        0, qci, per=2),
                          0, 4 * qci - 1)

            # ---------------- prologue ----------------
            # only what the first score steps need: kT block 0 first k-range
            # + qT block 0 first q-chunk
            for p in projT_parcels(kT[:, 0, :], wk_sb, bk_sb, xkv_ts,
                                   0, 0, per=4):
                p()
            for p in projT_parcels(qT_b[0], wq_sb, bq_sb, xq_ts, 0, 0, per=4):
                p()

            def norm_parcels(qci, pe_b, po_b, anorms):
                """normalize pair -> an tile, split into 3 parcels so the
                PE never waits on the DVE copies at a pair boundary.
                den row -> zeroed staging tile, selector matmul broadcasts it
                to 64 partitions, approx recip, multiply; the odd head is
                recombined into partitions 64..127 via an SBUF DMA."""
                cw = QC[qci][1]
                hold = {}

                def n1():
                    hold['an'] = an_pool.tile([128, 512], mt, tag="an", bufs=8,
                                              name="an")
                    nc.vector.tensor_copy(ds_e[64:65, :cw], pe_b[64:65, :cw])
                    hold['rb_e'] = aux_ps.tile([128, 512], F32, tag="aux",
                                               bufs=2, name="rb_e")
                    nc.tensor.matmul(
                        hold['rb_e'][0:64, :cw], sel_sb[:, :], ds_e[:, :cw],
                        start=True, stop=True,
                    )

                def n2():
                    rb_esb = small.tile([64, 512], F32, tag="rb", bufs=2,
                                        name="rb_esb")
                    nc.vector.reciprocal_approx_fast(rb_esb[:, :cw],
                                                     hold['rb_e'][0:64, :cw])
                    nc.vector.tensor_mul(
                        hold['an'][0:64, :cw], pe_b[0:64, :cw], rb_esb[:, :cw]
                    )
                    nc.vector.tensor_copy(ds_o[64:65, :cw], po_b[64:65, :cw])
                    hold['rb_o'] = aux_ps.tile([128, 512], F32, tag="aux",
                                               bufs=2, name="rb_o")
                    nc.tensor.matmul(
                        hold['rb_o'][0:64, :cw], sel_sb[:, :], ds_o[:, :cw],
                        start=True, stop=True,
                    )

                def n3():
                    rb_osb = small.tile([64, 512], F32, tag="rb", bufs=2,
                                        name="rb_osb")
                    nc.vector.reciprocal_approx_fast(rb_osb[:, :cw],
                                                     hold['rb_o'][0:64, :cw])
                    antmp = small.tile([64, 512], mt, tag="antmp", bufs=2)
                    nc.vector.tensor_mul(
                        antmp[:, :cw], po_b[0:64, :cw], rb_osb[:, :cw]
                    )
                    nc.sync.dma_start(hold['an'][64:128, :cw], antmp[:, :cw])
                    anorms.append(hold['an'])
                return [n1, n2, n3]

            # ---------------- main loop: flat 146-step pipeline ----------
            # scores/exp stream through all 144 (pair, ktile) steps with no
            # boundary pauses; attn@V trails by 2 steps across pair
            # boundaries; each pair's normalize is emitted right after its
            # last attn@V, behind the next pair's already-queued scores.
            NP_ALL = len(QC) * NPAIR
            anorms_of = [[] for _ in QC]
            oparts = {}
            pbuf = {}
            acc = {}
            for u in range(GTOT):
                if u < 12 * NP_ALL:
                    P, kt = divmod(u, 12)
                    qci, j = divmod(P, NPAIR)
                    qo, cw = QC[qci]
                    ko, kh = KT[kt]
                    st = st_ps.tile([128, 2, 512], F32, tag="st", bufs=2)
                    nc.tensor.matmul(
                        st[:kh, 0, :cw],
                        kT[0:64, j, ko:ko + kh],
                        qT_b[j][0:64, qo:qo + cw],
                        start=True, stop=True,
                    )
                    nc.tensor.matmul(
                        st[:kh, 1, :cw],
                        kT[64:128, j, ko:ko + kh],
                        qT_b[j][64:128, qo:qo + cw],
                        start=True, stop=True,
                    )
                    p = p_pool.tile([128, 2, 512], BF16, tag="p", bufs=20)
                    nc.scalar.activation(
                        p[:kh, :, :cw], st[:kh, :, :cw], AF.Exp,
                        scale=0.125,
                    )
                    pbuf[u] = p
                sched.on_step(u, budget=1)
                if u >= LAG:
                    P2, kc = divmod(u - LAG, 12)
                    qci2, j2 = divmod(P2, NPAIR)
                    qo2, cw2 = QC[qci2]
                    ko, kh = KT[kc]
                    if kc == 0:
                        acc[P2] = (
                            at_ps.tile([128, 512], F32, tag="attn", bufs=2,
                                       name=f"pe_{P2}"),
                            at_ps.tile([128, 512], F32, tag="attn", bufs=2,
                                       name=f"po_{P2}"),
                        )
                    pe_b, po_b = acc[P2]
                    nc.tensor.matmul(
                        pe_b[0:65, :cw2],
                        v_t[kc][0:kh, 2 * j2, :],
                        pbuf[u - LAG][0:kh, 0, :cw2],
                        start=(kc == 0), stop=(kc == NKT - 1),
                    )
                    nc.tensor.matmul(
                        po_b[0:65, :cw2],
                        v_t[kc][0:kh, 2 * j2 + 1, :],
                        pbuf[u - LAG][0:kh, 1, :cw2],
                        start=(kc == 0), stop=(kc == NKT - 1),
                    )
                    pbuf.pop(u - LAG)
                    if kc == NKT - 1:
                        # pair P2 complete: its normalize rides the filler
                        # stream; deadline = before pair P2+2 needs the
                        # accumulator banks back
                        sched.add(norm_parcels(qci2, pe_b, po_b,
                                               anorms_of[qci2]),
                                  u + 1, 12 * (P2 + 1) + LAG - 1)
                        acc.pop(P2)
                        if qci2 == len(QC) - 1 and j2 == 1:
                            sched.add(
                                list(outpart_parcels(qci2, anorms_of[qci2],
                                                     oparts)),
                                12 * (P2 + 2) + LAG - 1, GTOT - 8)
                        if j2 == NPAIR - 1 and qci2 < len(QC) - 1:
                            # chunk done: meter its out-projection into the
                            # next chunk's steps (after its last norm's dl)
                            sched.add(
                                outproj_parcels(qci2, anorms_of[qci2]),
                                12 * (P2 + 2) + LAG - 1,
                                48 * (qci2 + 2) - 10)

            # tail: last chunk's out-projection (pairs 2-3 only; pairs 0-1
            # were pre-accumulated into SBUF by outpart_parcels)
            sched.drain(GTOT)
            qo, cw = QC[-1]
            anorms = anorms_of[-1]
            nsub = (cw + 127) // 128
            for s in range(nsub):
                sw = min(128, cw - s * 128)
                osb = small.tile([128, 1024], F32, tag="os", bufs=2)
                for dc in range(2):
                    op = (st_ps if dc else aux_ps).tile(
                        [128, 512], F32, tag=("st" if dc else "aux"),
                        bufs=2, name=f"opt{dc}")
                    for jj in (2, 3):
                        nc.tensor.matmul(
                            op[:sw, :],
                            anorms[jj][:, s * 128:s * 128 + sw],
                            wo_sb[:, jj, dc * 512:(dc + 1) * 512],
                            start=(jj == 2), stop=(jj == NPAIR - 1),
                        )
                    nc.vector.tensor_add(
                        osb[:sw, dc * 512:(dc + 1) * 512],
                        op[:sw, :], oparts[(s, dc)][:sw, :])
                nc.sync.dma_start(
                    out_d[qo + s * 128:qo + s * 128 + sw, :], osb[:sw, :],
                )

    nc.compile()
    return nc


_NC = {}


def _get_nc(mode=MODE, with_bias=False):
    key = (mode, with_bias)
    if key not in _NC:
        _NC[key] = _build(mode, with_bias)
    return _NC[key]


def _pack_rows(a, nblk):
    """[nblk*128, N] -> [128, nblk*N]: partition p holds rows p, 128+p, ..."""
    n = a.shape[1]
    return np.ascontiguousarray(
        a.reshape(nblk, 128, n).transpose(1, 0, 2).reshape(128, nblk * n))


def _shard_inputs(mode, inputs_q, inputs_kv, Wq, bq, Wk, bk, Wv, bv, Wo, bo):
    ndt = ml_dtypes.bfloat16 if mode == "bf16" else np.float32
    in_maps = []
    for b in range(B):
        xqT = np.ascontiguousarray(inputs_q[b].T)
        xkvT = np.ascontiguousarray(inputs_kv[b].T)
        xqp = {f"xqp{qci}": _pack_rows(xqT[:, qo:qo + cwq], DCH).astype(ndt)
               for qci, (qo, cwq) in enumerate(QC)}
        xkvp = {f"xkvp{qci}": _pack_rows(xkvT[:, qo:qo + cwq], DCH).astype(ndt)
                for qci, (qo, cwq) in enumerate(QC)}
        for g in range(2):
            hs = slice(g * HG, (g + 1) * HG)
            wq = Wq[:, hs, :].reshape(D, HHD)
            wk = Wk[:, hs, :].reshape(D, HHD)
            wv = Wv[:, hs, :].reshape(D, HHD)
            # fuse wq|wk|wv: [128, c, 3, 512]
            wqkv = np.stack(
                [np.asarray(w).reshape(DCH, 128, HHD) for w in (wq, wk, wv)],
                axis=2,
            ).transpose(1, 0, 2, 3).reshape(128, DCH * 3 * HHD)
            in_maps.append({
                **xqp,
                **xkvp,
                "wqkv": np.ascontiguousarray(wqkv).astype(ndt),
                "wop": _pack_rows(np.asarray(Wo[hs].reshape(HHD, D)),
                                  HB).astype(ndt),
                "bq": np.ascontiguousarray(bq[hs].reshape(1, HHD)).astype(ndt),
                "bk": np.ascontiguousarray(bk[hs].reshape(1, HHD)).astype(ndt),
                "bv": np.ascontiguousarray(bv[hs].reshape(1, HHD)).astype(ndt),
            })
    return in_maps


def _run(inputs, trace=False, trace_kwargs=None, mode=MODE):
    inputs = {k: np.asarray(v) for k, v in inputs.items()}
    with_bias = bool(
        np.any(inputs["bq"]) or np.any(inputs["bk"]) or np.any(inputs["bv"])
    )
    nc = _get_nc(mode, with_bias)
    in_maps = _shard_inputs(mode, **inputs)
    res = run_bass_kernel_spmd(
        nc, in_maps, core_ids=list(range(2 * B)), trace=trace,
        **(trace_kwargs or {}),
    )
    bo = np.asarray(inputs["bo"], np.float32)
    out = np.empty((B, SEQ, D), np.float32)
    for b in range(B):
        out[b] = res.results[2 * b]["out"] + res.results[2 * b + 1]["out"] + bo
    return out, res


def kernel(**inputs):
    out, _ = _run(inputs, trace=False)
    return out
